# revision 43
# baseline (speedup 1.0000x reference)
"""Trainium2 Bass kernel for nn_BaselineTrustModel.

Math (see the reference): the per-timestep recurrence is affine and collapses
to a per-sample scalar formula.  With
    s    = sum_t perf[t, n]                (number of "fail" flags, 0..T)
    mask = any(obs[0, n, :] != 0)
    r1   = 1/sqrt(sigma0^2 + T*sigma_t^2)
    z0   = trust0/sqrt(sigma0^2)
    A    = (trust0 + T*wb + T*wtp) * r1
    B    = 2*wtp*r1
the output is
    pred[n] = clip(sigmoid(z0 + mask*( (A - z0) - B*s )), 0.01, 0.99)

Only obs[0] (N x D) and perf (T x N) are ever read -> ~66 MB of f32 input
traffic total, data-parallel over the sample axis N across 8 cores
(~8.3 MB per core, memory-bound; per-core HBM roofline ~358 GB/s -> ~23 us
of streaming; measured fixed preamble+tail of any NEFF here is ~13.5 us).

Device kernel per core (raw bacc, hand-scheduled; no TileContext).
Partition p owns samples [p*F, (p+1)*F), F = 490.  Every input tile is
resident in SBUF (8 MB total), so all 21 input DMAs are dispatched
back-to-back with no buffer-reuse gating; the two HWDGE queues stream at
full HBM rate for the whole kernel.

  SP  : obs chunks 0,1 then the 16 perf t-layer loads, then the 2 stores.
  ACT : obs chunks 2,3,4 (obs therefore finishes well before the perf
        stream), a table-prewarm sigmoid, then the 2 epilogue sigmoids.
  Q7  : accumulates perf layers 0..6 into sB (arrival-paced).
  DVE : accumulates layers 7..13 into sA, 5 segmented abs-max obs reduces,
        then s1=sA+sB, dd=s1*(-B)+(A-z0); layers 14/15 are folded straight
        into dd via scalar_tensor_tensor so the last-arriving bytes have the
        shortest possible tail; x=(ma>0)*dd; clip halves (pipelined with the
        ACT sigmoid halves and the 2 stores).
"""

import math
import sys
from contextlib import ExitStack

import numpy as np

for _p in ("/opt/trn_rl_repo", "/root/.axon_site/_ro/trn_rl_repo"):
    if _p not in sys.path:
        sys.path.append(_p)

T = 16
D = 16
N = 500000
NCORES = 8

F = 490            # samples per partition per core
K = 5              # obs chunks (F % K == 0)
MH = F // 2        # epilogue half width
NQ7 = 7            # perf layers 0..NQ7-1 summed by gpsimd
PER = 128 * F      # 62720 samples per core
NPAD = NCORES * PER


def build_program(neg_b, c_const, z0):
    """Raw-bacc single-core program (SPMD across cores)."""
    from concourse import bacc, mybir

    f32 = mybir.dt.float32
    fc = F // K                      # 98 samples per obs chunk per partition
    nc = bacc.Bacc("TRN2", target_bir_lowering=False, debug=False)
    obs_d = nc.dram_tensor("obs0", [128, K, fc * D], f32, kind="ExternalInput").ap()
    perf_d = nc.dram_tensor("perfc", [T, 128, F], f32, kind="ExternalInput").ap()
    out_d = nc.dram_tensor("out", [128, F], f32, kind="ExternalOutput").ap()

    with ExitStack() as ctx:
        sb = lambda name, shape: ctx.enter_context(nc.sbuf_tensor(name, shape, f32))
        pb = [sb(f"pb{i}", [128, F]) for i in range(T)]         # perf layers
        ob = [sb(f"ob{k}", [128, fc * D]) for k in range(K)]    # obs chunks
        sA = sb("sA", [128, F])
        sB = sb("sB", [128, F])
        s1 = sb("s1", [128, F])
        ma = sb("ma", [128, F])
        dd = sb("dd", [128, F])
        xx = sb("xx", [128, F])
        pp = sb("pp", [128, F])
        oo = sb("oo", [128, F])
        z0t = sb("z0t", [128, 1])
        scr = sb("scr", [128, 1])

        pdma = [ctx.enter_context(nc.semaphore(f"pd{i}")) for i in range(T)]
        obdma = [ctx.enter_context(nc.semaphore(f"od{k}")) for k in range(K)]
        odma = ctx.enter_context(nc.semaphore("odma"))
        dve = ctx.enter_context(nc.semaphore("dve"))
        q7 = ctx.enter_context(nc.semaphore("q7"))
        act = ctx.enter_context(nc.semaphore("act"))
        all_sems = pdma + obdma + [odma, dve, q7, act]
        nums = sorted(s.num for s in all_sems)
        assert nums == list(range(nums[0], nums[0] + len(nums))), nums
        sem_range = range(nums[0], nums[-1] + 1)

        # DVE op counter values (op i completes -> dve == i):
        #  1 memset | 2 r0 | 3 r1 | 4 sA=l7+l8 | 5 r2 | 6..8 +=l9,l10,l11 |
        #  9 r3 | 10,11 +=l12,l13 | 12 r4 | 13 s1 | 14 dd | 15 +=l14 |
        #  16 +=l15 | 17 x | 18 clip_h0 | 19 clip_h1
        X_N, CLIP0_N, CLIP1_N = 17, 18, 19

        block_cm = nc.Block()
        block = block_cm.__enter__()

        @block.sync
        def _(sync):
            for k in (0, 1):
                sync.dma_start(ob[k][:], obs_d[:, k]).then_inc(obdma[k], 16)
            for i in range(T):
                sync.dma_start(pb[i][:], perf_d[i]).then_inc(pdma[i], 16)
            sync.wait_ge(dve, CLIP0_N)
            sync.dma_start(out_d[:, 0:MH], oo[:, 0:MH]).then_inc(odma, 16)
            sync.wait_ge(dve, CLIP1_N)
            sync.dma_start(out_d[:, MH:F], oo[:, MH:F]).then_inc(odma, 16)
            sync.wait_ge(odma, 32)

        @block.scalar
        def _(scalar):
            for k in (2, 3, 4):
                scalar.dma_start(ob[k][:], obs_d[:, k]).then_inc(obdma[k], 16)
            # prewarm the sigmoid table set while the stream runs
            scalar.wait_ge(dve, 1)
            nc.scalar.activation(
                scr[:], z0t[:], mybir.ActivationFunctionType.Sigmoid,
            ).then_inc(act, 1)
            scalar.wait_ge(dve, X_N)
            nc.scalar.activation(
                pp[:, 0:MH], xx[:, 0:MH], mybir.ActivationFunctionType.Sigmoid,
                bias=z0t[:], scale=1.0,
            ).then_inc(act, 1)
            nc.scalar.activation(
                pp[:, MH:F], xx[:, MH:F], mybir.ActivationFunctionType.Sigmoid,
                bias=z0t[:], scale=1.0,
            ).then_inc(act, 1)

        @block.gpsimd
        def _(gpsimd):
            gpsimd.wait_ge(pdma[1], 16)
            gpsimd.wait_ge(pdma[0], 16)
            nc.gpsimd.tensor_add(sB[:], pb[0][:], pb[1][:]).then_inc(q7, 1)
            for i in range(2, NQ7):
                gpsimd.wait_ge(pdma[i], 16)
                gpsimd.wait_ge(q7, i - 1)  # RAW sB write-back
                nc.gpsimd.tensor_add(sB[:], sB[:], pb[i][:]).then_inc(q7, 1)

        @block.vector
        def _(vector):
            cnt = [0]

            def emit(instr):
                instr.then_inc(dve, 1)
                cnt[0] += 1
                return cnt[0]

            def reduce(k):
                vector.wait_ge(obdma[k], 16)
                emit(nc.vector.tensor_reduce(
                    ma[:, k * fc:(k + 1) * fc],
                    ob[k][:].rearrange("p (f d) -> p f d", d=D),
                    axis=mybir.AxisListType.X,
                    op=mybir.AluOpType.max,
                    apply_absolute_value=True,
                ))

            emit(nc.vector.memset(z0t[:], z0))
            reduce(0)
            reduce(1)
            vector.wait_ge(pdma[7], 16)
            vector.wait_ge(pdma[8], 16)
            emit(nc.vector.tensor_add(sA[:], pb[7][:], pb[8][:]))
            reduce(2)
            for i in (9, 10, 11):
                vector.wait_ge(pdma[i], 16)
                vector.wait_ge(dve, cnt[0])  # RAW sA write-back
                emit(nc.vector.tensor_add(sA[:], sA[:], pb[i][:]))
            reduce(3)
            for i in (12, 13):
                vector.wait_ge(pdma[i], 16)
                vector.wait_ge(dve, cnt[0])
                emit(nc.vector.tensor_add(sA[:], sA[:], pb[i][:]))
            reduce(4)
            vector.wait_ge(q7, NQ7 - 1)
            vector.wait_ge(dve, cnt[0])
            emit(nc.vector.tensor_add(s1[:], sA[:], sB[:]))
            vector.wait_ge(dve, cnt[0])
            emit(nc.vector.tensor_scalar(
                dd[:], s1[:], neg_b, c_const,
                op0=mybir.AluOpType.mult, op1=mybir.AluOpType.add,
            ))
            for i in (14, 15):
                vector.wait_ge(pdma[i], 16)
                vector.wait_ge(dve, cnt[0])
                emit(nc.vector.scalar_tensor_tensor(
                    dd[:], pb[i][:], neg_b, dd[:],
                    op0=mybir.AluOpType.mult, op1=mybir.AluOpType.add,
                ))
            vector.wait_ge(dve, cnt[0])
            emit(nc.vector.scalar_tensor_tensor(
                xx[:], ma[:], 0.0, dd[:],
                op0=mybir.AluOpType.is_gt, op1=mybir.AluOpType.mult,
            ))
            assert cnt[0] == X_N, cnt[0]
            for h in range(2):
                vector.wait_ge(act, h + 2)
                emit(nc.vector.tensor_scalar(
                    oo[:, h * MH:(h + 1) * MH], pp[:, h * MH:(h + 1) * MH],
                    0.01, 0.99,
                    op0=mybir.AluOpType.max, op1=mybir.AluOpType.min,
                ))
            assert cnt[0] == CLIP1_N, cnt[0]

        block_cm.__exit__(None, None, None)
        # Re-executable NEFF tail (the NTFF profiler replays it).
        nc.all_engine_barrier()
        nc.gpsimd.dma_reset(sem_range)
        nc.gpsimd.sem_clear(sem_range)

    nc.compile()
    return nc


def _scalar_constants(inputs):
    t0 = float(np.asarray(inputs["trust0"]).reshape(()))
    s0 = float(np.asarray(inputs["sigma0"]).reshape(()))
    wb = float(np.asarray(inputs["wb"]).reshape(()))
    wtp = float(np.asarray(inputs["wtp"]).reshape(()))
    st = float(np.asarray(inputs["sigma_t"]).reshape(()))
    r1 = 1.0 / math.sqrt(s0 * s0 + T * st * st)
    z0 = t0 / math.sqrt(s0 * s0)
    a_const = (t0 + T * wb + T * wtp) * r1
    neg_b = -2.0 * wtp * r1
    c_const = a_const - z0
    return neg_b, c_const, z0


def run(inputs, trace=False, **kw):
    """Shard, run on 8 cores, gather. Returns (output [N,1] f32, exec_time_ns)."""
    from concourse.bass_utils import run_bass_kernel_spmd

    obs = np.asarray(inputs["inptasksobs"])
    perf = np.asarray(inputs["inptasksperf"])
    assert obs.shape == (T, N, D) and perf.shape == (T, N, 1)

    neg_b, c_const, z0 = _scalar_constants(inputs)
    nc = build_program(neg_b, c_const, z0)

    obs_p = np.zeros((NPAD, D), np.float32)
    obs_p[:N] = obs[0]
    perf_p = np.zeros((T, NPAD), np.float32)
    perf_p[:, :N] = perf[:, :, 0]

    in_maps = []
    for c in range(NCORES):
        oc = obs_p[c * PER:(c + 1) * PER].reshape(128, K, (F // K) * D)
        pc = np.ascontiguousarray(
            perf_p[:, c * PER:(c + 1) * PER]
        ).reshape(T, 128, F)
        in_maps.append({"obs0": oc, "perfc": pc})

    res = run_bass_kernel_spmd(
        nc, in_maps, core_ids=list(range(NCORES)), trace=trace, **kw
    )
    full = np.concatenate(
        [res.results[c]["out"].reshape(-1) for c in range(NCORES)]
    )
    return full[:N].reshape(N, 1).astype(np.float32, copy=False), res.exec_time_ns


def kernel(**inputs):
    out, _ = run(inputs, trace=False)
    return out


# revision 44
# speedup vs baseline: 1.1028x; 1.1028x over previous
"""Trainium2 Bass kernel for nn_BaselineTrustModel.

Math (see the reference): the per-timestep recurrence is affine and collapses
to a per-sample scalar formula.  With
    s    = sum_t perf[t, n]                (number of "fail" flags, 0..T)
    mask = any(obs[0, n, :] != 0)
    r1   = 1/sqrt(sigma0^2 + T*sigma_t^2)
    z0   = trust0/sqrt(sigma0^2)
    A    = (trust0 + T*wb + T*wtp) * r1
    B    = 2*wtp*r1
the output is
    pred[n] = clip(sigmoid(z0 + mask*( (A - z0) - B*s )), 0.01, 0.99)

Only obs[0] (N x D) and perf (T x N) are ever read -> ~66 MB of f32 input
traffic total, data-parallel over the sample axis N across 8 cores
(~8.3 MB per core, memory-bound; per-core HBM roofline ~358 GB/s -> ~23 us
of streaming; measured fixed preamble+tail of any NEFF here is ~13.5 us).

Device kernel per core (raw bacc, hand-scheduled; no TileContext).
Partition p owns samples [p*F, (p+1)*F), F = 490.  All input tiles are
SBUF-resident (8 MB), every DMA is dispatched with no buffer-reuse gating.
HWDGE queues are descriptor-rate-bound (~13 ns/desc) for the 1960 B-packet
perf layers, so the 16 layer loads are split across BOTH queues,
interleaved with the obs chunks; the last two layers are folded directly
into the affine epilogue (scalar_tensor_tensor) so the last-arriving bytes
have the shortest tail.  gpsimd does no elementwise work (it port-shares
with and degrades the vector engine).

  SP  : k0, l0, l2, k2, l4, l6, l8, l10, l12, l14, stores
  ACT : k1, l1, k3, l3, l5, k4, l7, l9, l11, l13, l15, prewarm + sigmoids
  DVE : sA = l0+...+l13 (arrival-paced), 5 segmented abs-max reduces,
        dd = sA*(-B)+(A-z0), dd += l14*(-B), dd += l15*(-B),
        x = (ma>0)*dd, clip halves (pipelined with ACT sigmoid halves).
"""

import math
import sys
from contextlib import ExitStack

import numpy as np

for _p in ("/opt/trn_rl_repo", "/root/.axon_site/_ro/trn_rl_repo"):
    if _p not in sys.path:
        sys.path.append(_p)

T = 16
D = 16
N = 500000
NCORES = 8

F = 490            # samples per partition per core
K = 5              # obs chunks (F % K == 0)
MH = F // 2        # epilogue half width
PER = 128 * F      # 62720 samples per core
NPAD = NCORES * PER

SP_ORDER = ["k0", "l0", "l2", "k2", "l4", "l6", "l8", "l10", "l12", "l14"]
ACT_ORDER = ["k1", "l1", "k3", "l3", "l5", "k4", "l7", "l9", "l11", "l13", "l15"]


def build_program(neg_b, c_const, z0):
    """Raw-bacc single-core program (SPMD across cores)."""
    from concourse import bacc, mybir

    f32 = mybir.dt.float32
    fc = F // K                      # 98 samples per obs chunk per partition
    nc = bacc.Bacc("TRN2", target_bir_lowering=False, debug=False)
    obs_d = nc.dram_tensor("obs0", [128, K, fc * D], f32, kind="ExternalInput").ap()
    perf_d = nc.dram_tensor("perfc", [T, 128, F], f32, kind="ExternalInput").ap()
    out_d = nc.dram_tensor("out", [128, F], f32, kind="ExternalOutput").ap()

    with ExitStack() as ctx:
        sb = lambda name, shape: ctx.enter_context(nc.sbuf_tensor(name, shape, f32))
        pb = [sb(f"pb{i}", [128, F]) for i in range(T)]         # perf layers
        ob = [sb(f"ob{k}", [128, fc * D]) for k in range(K)]    # obs chunks
        sA = sb("sA", [128, F])
        ma = sb("ma", [128, F])
        dd = sb("dd", [128, F])
        xx = sb("xx", [128, F])
        pp = sb("pp", [128, F])
        oo = sb("oo", [128, F])
        z0t = sb("z0t", [128, 1])
        scr = sb("scr", [128, 1])

        pdma = [ctx.enter_context(nc.semaphore(f"pd{i}")) for i in range(T)]
        obdma = [ctx.enter_context(nc.semaphore(f"od{k}")) for k in range(K)]
        odma = ctx.enter_context(nc.semaphore("odma"))
        dve = ctx.enter_context(nc.semaphore("dve"))
        act = ctx.enter_context(nc.semaphore("act"))
        all_sems = pdma + obdma + [odma, dve, act]
        nums = sorted(s.num for s in all_sems)
        assert nums == list(range(nums[0], nums[0] + len(nums))), nums
        sem_range = range(nums[0], nums[-1] + 1)

        block_cm = nc.Block()
        block = block_cm.__enter__()

        marks = {}  # landmark name -> dve counter value

        @block.vector
        def _(vector):
            cnt = [0]

            def emit(instr, mark=None):
                instr.then_inc(dve, 1)
                cnt[0] += 1
                if mark:
                    marks[mark] = cnt[0]
                return cnt[0]

            def reduce(k):
                vector.wait_ge(obdma[k], 16)
                emit(nc.vector.tensor_reduce(
                    ma[:, k * fc:(k + 1) * fc],
                    ob[k][:].rearrange("p (f d) -> p f d", d=D),
                    axis=mybir.AxisListType.X,
                    op=mybir.AluOpType.max,
                    apply_absolute_value=True,
                ))

            emit(nc.vector.memset(z0t[:], z0), mark="z0")
            reduce(0)
            vector.wait_ge(pdma[0], 16)
            vector.wait_ge(pdma[1], 16)
            emit(nc.vector.tensor_add(sA[:], pb[0][:], pb[1][:]))
            # arrival-interleaved: adds l2..l13, reduces r1..r4
            sched = ["l2", "r1", "l3", "l4", "r2", "l5", "l6", "l7",
                     "r3", "l8", "l9", "l10", "r4", "l11", "l12", "l13"]
            for tok in sched:
                if tok[0] == "r":
                    reduce(int(tok[1:]))
                else:
                    i = int(tok[1:])
                    vector.wait_ge(pdma[i], 16)
                    vector.wait_ge(dve, cnt[0])  # RAW sA write-back
                    emit(nc.vector.tensor_add(sA[:], sA[:], pb[i][:]))
            vector.wait_ge(dve, cnt[0])
            emit(nc.vector.tensor_scalar(
                dd[:], sA[:], neg_b, c_const,
                op0=mybir.AluOpType.mult, op1=mybir.AluOpType.add,
            ))
            for i in (14, 15):
                vector.wait_ge(pdma[i], 16)
                vector.wait_ge(dve, cnt[0])
                emit(nc.vector.scalar_tensor_tensor(
                    dd[:], pb[i][:], neg_b, dd[:],
                    op0=mybir.AluOpType.mult, op1=mybir.AluOpType.add,
                ))
            vector.wait_ge(dve, cnt[0])
            emit(nc.vector.scalar_tensor_tensor(
                xx[:], ma[:], 0.0, dd[:],
                op0=mybir.AluOpType.is_gt, op1=mybir.AluOpType.mult,
            ), mark="x")
            for h in range(2):
                vector.wait_ge(act, h + 2)
                emit(nc.vector.tensor_scalar(
                    oo[:, h * MH:(h + 1) * MH], pp[:, h * MH:(h + 1) * MH],
                    0.01, 0.99,
                    op0=mybir.AluOpType.max, op1=mybir.AluOpType.min,
                ), mark=f"clip{h}")

        def dma_item(eng, tok):
            if tok[0] == "k":
                k = int(tok[1:])
                eng.dma_start(ob[k][:], obs_d[:, k]).then_inc(obdma[k], 16)
            else:
                i = int(tok[1:])
                eng.dma_start(pb[i][:], perf_d[i]).then_inc(pdma[i], 16)

        @block.sync
        def _(sync):
            for tok in SP_ORDER:
                dma_item(sync, tok)
            sync.wait_ge(dve, marks["clip0"])
            sync.dma_start(out_d[:, 0:MH], oo[:, 0:MH]).then_inc(odma, 16)
            sync.wait_ge(dve, marks["clip1"])
            sync.dma_start(out_d[:, MH:F], oo[:, MH:F]).then_inc(odma, 16)
            sync.wait_ge(odma, 32)

        @block.scalar
        def _(scalar):
            for tok in ACT_ORDER:
                dma_item(scalar, tok)
            # prewarm the sigmoid table set while the stream runs
            scalar.wait_ge(dve, marks["z0"])
            nc.scalar.activation(
                scr[:], z0t[:], mybir.ActivationFunctionType.Sigmoid,
            ).then_inc(act, 1)
            scalar.wait_ge(dve, marks["x"])
            for h in range(2):
                nc.scalar.activation(
                    pp[:, h * MH:(h + 1) * MH], xx[:, h * MH:(h + 1) * MH],
                    mybir.ActivationFunctionType.Sigmoid,
                    bias=z0t[:], scale=1.0,
                ).then_inc(act, 1)

        block_cm.__exit__(None, None, None)
        # Re-executable NEFF tail (the NTFF profiler replays it).
        nc.all_engine_barrier()
        nc.gpsimd.dma_reset(sem_range)
        nc.gpsimd.sem_clear(sem_range)

    nc.compile()
    return nc


def _scalar_constants(inputs):
    t0 = float(np.asarray(inputs["trust0"]).reshape(()))
    s0 = float(np.asarray(inputs["sigma0"]).reshape(()))
    wb = float(np.asarray(inputs["wb"]).reshape(()))
    wtp = float(np.asarray(inputs["wtp"]).reshape(()))
    st = float(np.asarray(inputs["sigma_t"]).reshape(()))
    r1 = 1.0 / math.sqrt(s0 * s0 + T * st * st)
    z0 = t0 / math.sqrt(s0 * s0)
    a_const = (t0 + T * wb + T * wtp) * r1
    neg_b = -2.0 * wtp * r1
    c_const = a_const - z0
    return neg_b, c_const, z0


def run(inputs, trace=False, **kw):
    """Shard, run on 8 cores, gather. Returns (output [N,1] f32, exec_time_ns)."""
    from concourse.bass_utils import run_bass_kernel_spmd

    obs = np.asarray(inputs["inptasksobs"])
    perf = np.asarray(inputs["inptasksperf"])
    assert obs.shape == (T, N, D) and perf.shape == (T, N, 1)

    neg_b, c_const, z0 = _scalar_constants(inputs)
    nc = build_program(neg_b, c_const, z0)

    obs_p = np.zeros((NPAD, D), np.float32)
    obs_p[:N] = obs[0]
    perf_p = np.zeros((T, NPAD), np.float32)
    perf_p[:, :N] = perf[:, :, 0]

    in_maps = []
    for c in range(NCORES):
        oc = obs_p[c * PER:(c + 1) * PER].reshape(128, K, (F // K) * D)
        pc = np.ascontiguousarray(
            perf_p[:, c * PER:(c + 1) * PER]
        ).reshape(T, 128, F)
        in_maps.append({"obs0": oc, "perfc": pc})

    res = run_bass_kernel_spmd(
        nc, in_maps, core_ids=list(range(NCORES)), trace=trace, **kw
    )
    full = np.concatenate(
        [res.results[c]["out"].reshape(-1) for c in range(NCORES)]
    )
    return full[:N].reshape(N, 1).astype(np.float32, copy=False), res.exec_time_ns


def kernel(**inputs):
    out, _ = run(inputs, trace=False)
    return out


# revision 46
# speedup vs baseline: 1.1857x; 1.0752x over previous
"""Trainium2 Bass kernel for nn_BaselineTrustModel.

Math (see the reference): the per-timestep recurrence is affine and collapses
to a per-sample scalar formula.  With
    s    = sum_t perf[t, n]                (number of "fail" flags, 0..T)
    mask = any(obs[0, n, :] != 0)
    r1   = 1/sqrt(sigma0^2 + T*sigma_t^2)
    z0   = trust0/sqrt(sigma0^2)
    A    = (trust0 + T*wb + T*wtp) * r1
    B    = 2*wtp*r1
the output is
    pred[n] = clip(sigmoid(z0 + mask*( (A - z0) - B*s )), 0.01, 0.99)

Only obs[0] (N x D) and perf (T x N) are ever read -> ~66 MB of f32 input
traffic total, data-parallel over the sample axis N across 8 cores
(~8.3 MB per core, memory-bound; per-core HBM roofline ~358 GB/s -> ~23 us
of streaming; measured fixed preamble+tail of any NEFF here is ~13.5 us).

Device kernel per core (raw bacc, hand-scheduled; no TileContext).
Partition p owns samples [p*F, (p+1)*F), F = 490.  All tiles SBUF-resident;
every DMA dispatched with no buffer-reuse gating.  Engine split:

  Q7  : 16 perf t-layer cast-DMAs (SWDGE, f32 DRAM -> bf16 SBUF; perf
        values are 0/1 so the cast is exact).  SWDGE lanes add descriptor
        bandwidth alongside the two HWDGE queues.
  SP  : identity load + obs chunks 0,2,4 (HWDGE), the 2 stores.
  ACT : obs chunks 1,3 (its own HWDGE queue), table prewarm + 2 sigmoids.
  PE  : s = sum_t perf[t] as 16 PSUM-accumulated identity matmuls
        (I.T @ l_t accumulated; bf16 x bf16 -> f32 PSUM, exact).
  DVE : 5 segmented abs-max obs reduces, dd = s*(-B)+(A-z0) straight from
        PSUM, x = (ma>0)*dd, clip halves (pipelined with ACT sigmoids).
"""

import math
import sys
from contextlib import ExitStack

import numpy as np

for _p in ("/opt/trn_rl_repo", "/root/.axon_site/_ro/trn_rl_repo"):
    if _p not in sys.path:
        sys.path.append(_p)

T = 16
D = 16
N = 500000
NCORES = 8

F = 490            # samples per partition per core
K = 5              # obs chunks (F % K == 0)
MH = F // 2        # epilogue half width
PER = 128 * F      # 62720 samples per core
NPAD = NCORES * PER


def build_program(neg_b, c_const, z0):
    """Raw-bacc single-core program (SPMD across cores)."""
    from concourse import bacc, mybir

    f32 = mybir.dt.float32
    bf16 = mybir.dt.bfloat16
    fc = F // K                      # 98 samples per obs chunk per partition
    nc = bacc.Bacc("TRN2", target_bir_lowering=False, debug=False)
    obs_d = nc.dram_tensor("obs0", [128, K, fc * D], f32, kind="ExternalInput").ap()
    perf_d = nc.dram_tensor("perfc", [T, 128, F], f32, kind="ExternalInput").ap()
    id_d = nc.dram_tensor("ident", [128, 128], bf16, kind="ExternalInput").ap()
    out_d = nc.dram_tensor("out", [128, F], f32, kind="ExternalOutput").ap()

    with ExitStack() as ctx:
        pb = [
            ctx.enter_context(nc.sbuf_tensor(f"pb{i}", [128, F], bf16))
            for i in range(T)
        ]
        sbf = lambda name, shape: ctx.enter_context(nc.sbuf_tensor(name, shape, f32))
        ob = [sbf(f"ob{k}", [128, fc * D]) for k in range(K)]
        ident = ctx.enter_context(nc.sbuf_tensor("idnt", [128, 128], bf16))
        ma = sbf("ma", [128, F])
        dd = sbf("dd", [128, F])
        xx = sbf("xx", [128, F])
        pp = sbf("pp", [128, F])
        oo = sbf("oo", [128, F])
        z0t = sbf("z0t", [128, 1])
        scr = sbf("scr", [128, 1])
        ps = ctx.enter_context(nc.psum_tensor("ps", [128, F], f32))

        pdma = [ctx.enter_context(nc.semaphore(f"pd{i}")) for i in range(T)]
        obdma = [ctx.enter_context(nc.semaphore(f"od{k}")) for k in range(K)]
        iddma = ctx.enter_context(nc.semaphore("iddma"))
        odma = ctx.enter_context(nc.semaphore("odma"))
        dve = ctx.enter_context(nc.semaphore("dve"))
        pe = ctx.enter_context(nc.semaphore("pe"))
        act = ctx.enter_context(nc.semaphore("act"))
        all_sems = pdma + obdma + [iddma, odma, dve, pe, act]
        nums = sorted(s.num for s in all_sems)
        assert nums == list(range(nums[0], nums[0] + len(nums))), nums
        sem_range = range(nums[0], nums[-1] + 1)

        block_cm = nc.Block()
        block = block_cm.__enter__()

        marks = {}  # landmark name -> dve counter value

        @block.gpsimd
        def _(gpsimd):
            for i in range(T):
                gpsimd.dma_start(pb[i][:], perf_d[i]).then_inc(pdma[i], 16)

        @block.tensor
        def _(tensor):
            tensor.wait_ge(iddma, 16)
            for i in range(T):
                tensor.wait_ge(pdma[i], 16)
                nc.tensor.matmul(
                    ps[:], ident[:], pb[i][:],
                    start=(i == 0), stop=(i == T - 1),
                ).then_inc(pe, 1)

        @block.vector
        def _(vector):
            cnt = [0]

            def emit(instr, mark=None):
                instr.then_inc(dve, 1)
                cnt[0] += 1
                if mark:
                    marks[mark] = cnt[0]
                return cnt[0]

            emit(nc.vector.memset(z0t[:], z0), mark="z0")
            for k in range(K):
                vector.wait_ge(obdma[k], 16)
                emit(nc.vector.tensor_reduce(
                    ma[:, k * fc:(k + 1) * fc],
                    ob[k][:].rearrange("p (f d) -> p f d", d=D),
                    axis=mybir.AxisListType.X,
                    op=mybir.AluOpType.max,
                    apply_absolute_value=True,
                ))
            vector.wait_ge(pe, T)
            emit(nc.vector.tensor_scalar(
                dd[:], ps[:], neg_b, c_const,
                op0=mybir.AluOpType.mult, op1=mybir.AluOpType.add,
            ))
            vector.wait_ge(dve, cnt[0])
            emit(nc.vector.scalar_tensor_tensor(
                xx[:], ma[:], 0.0, dd[:],
                op0=mybir.AluOpType.is_gt, op1=mybir.AluOpType.mult,
            ), mark="x")
            for h in range(2):
                vector.wait_ge(act, h + 2)
                emit(nc.vector.tensor_scalar(
                    oo[:, h * MH:(h + 1) * MH], pp[:, h * MH:(h + 1) * MH],
                    0.01, 0.99,
                    op0=mybir.AluOpType.max, op1=mybir.AluOpType.min,
                ), mark=f"clip{h}")

        @block.sync
        def _(sync):
            sync.dma_start(ident[:], id_d).then_inc(iddma, 16)
            for k in (0, 2, 4):
                sync.dma_start(ob[k][:], obs_d[:, k]).then_inc(obdma[k], 16)
            sync.wait_ge(dve, marks["clip0"])
            sync.dma_start(out_d[:, 0:MH], oo[:, 0:MH]).then_inc(odma, 16)
            sync.wait_ge(dve, marks["clip1"])
            sync.dma_start(out_d[:, MH:F], oo[:, MH:F]).then_inc(odma, 16)
            sync.wait_ge(odma, 32)

        @block.scalar
        def _(scalar):
            for k in (1, 3):
                scalar.dma_start(ob[k][:], obs_d[:, k]).then_inc(obdma[k], 16)
            # prewarm the sigmoid table set while the stream runs
            scalar.wait_ge(dve, marks["z0"])
            nc.scalar.activation(
                scr[:], z0t[:], mybir.ActivationFunctionType.Sigmoid,
            ).then_inc(act, 1)
            scalar.wait_ge(dve, marks["x"])
            for h in range(2):
                nc.scalar.activation(
                    pp[:, h * MH:(h + 1) * MH], xx[:, h * MH:(h + 1) * MH],
                    mybir.ActivationFunctionType.Sigmoid,
                    bias=z0t[:], scale=1.0,
                ).then_inc(act, 1)

        block_cm.__exit__(None, None, None)
        # Re-executable NEFF tail (the NTFF profiler replays it).
        nc.all_engine_barrier()
        nc.gpsimd.dma_reset(sem_range)
        nc.gpsimd.sem_clear(sem_range)

    nc.compile()
    return nc


def _scalar_constants(inputs):
    t0 = float(np.asarray(inputs["trust0"]).reshape(()))
    s0 = float(np.asarray(inputs["sigma0"]).reshape(()))
    wb = float(np.asarray(inputs["wb"]).reshape(()))
    wtp = float(np.asarray(inputs["wtp"]).reshape(()))
    st = float(np.asarray(inputs["sigma_t"]).reshape(()))
    r1 = 1.0 / math.sqrt(s0 * s0 + T * st * st)
    z0 = t0 / math.sqrt(s0 * s0)
    a_const = (t0 + T * wb + T * wtp) * r1
    neg_b = -2.0 * wtp * r1
    c_const = a_const - z0
    return neg_b, c_const, z0


def run(inputs, trace=False, **kw):
    """Shard, run on 8 cores, gather. Returns (output [N,1] f32, exec_time_ns)."""
    import ml_dtypes
    from concourse.bass_utils import run_bass_kernel_spmd

    obs = np.asarray(inputs["inptasksobs"])
    perf = np.asarray(inputs["inptasksperf"])
    assert obs.shape == (T, N, D) and perf.shape == (T, N, 1)

    neg_b, c_const, z0 = _scalar_constants(inputs)
    nc = build_program(neg_b, c_const, z0)

    obs_p = np.zeros((NPAD, D), np.float32)
    obs_p[:N] = obs[0]
    perf_p = np.zeros((T, NPAD), np.float32)
    perf_p[:, :N] = perf[:, :, 0]
    ident = np.eye(128, dtype=ml_dtypes.bfloat16)

    in_maps = []
    for c in range(NCORES):
        oc = obs_p[c * PER:(c + 1) * PER].reshape(128, K, (F // K) * D)
        pc = np.ascontiguousarray(
            perf_p[:, c * PER:(c + 1) * PER]
        ).reshape(T, 128, F)
        in_maps.append({"obs0": oc, "perfc": pc, "ident": ident})

    res = run_bass_kernel_spmd(
        nc, in_maps, core_ids=list(range(NCORES)), trace=trace, **kw
    )
    full = np.concatenate(
        [res.results[c]["out"].reshape(-1) for c in range(NCORES)]
    )
    return full[:N].reshape(N, 1).astype(np.float32, copy=False), res.exec_time_ns


def kernel(**inputs):
    out, _ = run(inputs, trace=False)
    return out
